# revision 27
# baseline (speedup 1.0000x reference)
"""Multi-head cross-attention Trainium2 kernel.

Full-input contract: kernel(**inputs) takes the complete tensors and returns
the complete output. Internally shards over 8 NeuronCores as
(batch x head-group): core c handles batch c//4 and heads [4*(c%4), 4*(c%4)+4).
Each core computes its partial output  ctx_g @ Wo_g  for its batch; the host
sums the 4 head-group partials per batch and adds bo.

Masked keys (key_mask == 0) contribute exactly zero probability, so the host
compacts key/value to the unmasked rows (padded up to a multiple of 128 with
-1e9 score bias), which shrinks the K/V projections and the whole attention
core proportionally.

Numerics: HBM-resident activations and projection weights are bf16 (halves
the DMA-bound), everything on-chip is fp32 (fp32r for matmul operands, which
streams at full rate for moving dims >= 256). Exact algebraic cuts:
  - bk is dropped entirely: a per-query constant added to every key's score
    is softmax-invariant.
  - bv is folded into bo on the host: probs sum to 1, so ctx = P@(v + bv) =
    P@v + bv, hence out += bv @ Wo, absorbed into bo.
  - bq is applied on-device in the Q projection.

Engines execute their queues in emission order, so the emission is software
pipelined (PSUM is the scarce resource - 8 banks):
  P1  prologue: block-0 K projection + Q projection.  Then heads 0,1
      attention per 128-key tile; the remaining K/V projection matmul groups
      (value-chunk-stationary V proj lands directly in [sk, d] layout - no PE
      transpose) are spread between dependent score/exp/ctx ops as PE filler
      to hide the ~800ns PSUM-drain+semaphore latency around each exp.
  P2a head 2 attention (one 1024-wide exp per key tile, ctx lagging 2 units
      so the pipeline is ACT-rate-limited), interleaved with the t=0 half of
      the output projection (heads 0,1 context ready) into 2 spare PSUM
      banks, staged to SBUF in fp32.
  P2b head 3 attention, with head-2 normalization folded in.
  P3  out = t=1 half + staged t=0 half, written back as bf16; host upcasts
      and reduces.
"""

import numpy as np
import ml_dtypes

B, SQ, SK, IN = 2, 1024, 4096, 1024
H_TOT, D, HPC = 16, 64, 4
DH = HPC * D  # 256, per-core head-dim slice
NCORES = 8

_CACHE = {}

# scheduling knobs
CFG = {"sin": 4, "sexp": 4, "sout": 3}


def _build(skp):
    import concourse.tile as tile
    from concourse import bacc, mybir

    FP = mybir.dt.float32
    FR = mybir.dt.float32r
    BF = mybir.dt.bfloat16
    AF = mybir.ActivationFunctionType
    MUL = mybir.AluOpType.mult
    ADD = mybir.AluOpType.add

    nc = bacc.Bacc("TRN2", target_bir_lowering=False, debug=False)

    NSKT = skp // 128          # sk tiles of 128
    NKC = IN // 128            # 8 contraction chunks
    SCALE = 1.0 / float(np.sqrt(D))

    qT_d = nc.dram_tensor("qT", [IN, SQ], BF, kind="ExternalInput").ap()
    kx_d = nc.dram_tensor("kx", [IN, skp], BF, kind="ExternalInput").ap()
    vx_d = nc.dram_tensor("vx", [IN, skp], BF, kind="ExternalInput").ap()
    wk_d = nc.dram_tensor("wk", [IN, DH], BF, kind="ExternalInput").ap()
    wqv_d = nc.dram_tensor("wqv", [IN, 2 * DH], BF, kind="ExternalInput").ap()
    wo_d = nc.dram_tensor("wo", [DH, SQ], FR, kind="ExternalInput").ap()
    # packed per-partition constants: cols [0:NSKT) mask bias, [NSKT:NSKT+2) bq
    mbq_d = nc.dram_tensor("mbq", [128, NSKT + 2], FP, kind="ExternalInput").ap()
    ones_d = nc.dram_tensor("ones", [1, 128], FR, kind="ExternalInput").ap()
    idn_d = nc.dram_tensor("idn", [128, 128], BF, kind="ExternalInput").ap()
    out_d = nc.dram_tensor("out", [SQ, SQ], BF, kind="ExternalOutput").ap()

    def blocks_of(width, step=512):
        out, off = [], 0
        while off < width:
            w = min(step, width - off)
            out.append((off, w))
            off += w
        return out

    blocks = blocks_of(skp)

    with tile.TileContext(nc) as tc:
        # ---- resident tensors (one bufs=1 pool; distinct names = own slots) ----
        cpool_cm = tc.tile_pool(name="const", bufs=1)
        cpool = cpool_cm.__enter__()
        wk_sb = cpool.tile([128, NKC, DH], BF, name="wk_sb")
        wqv_sb = cpool.tile([128, NKC, 2 * DH], BF, name="wqv_sb")
        qx_sb = cpool.tile([128, NKC, SQ], BF, name="qx_sb")
        idn_sb = cpool.tile([128, 128], BF, name="idn_sb")
        wo_sb = cpool.tile([128, 2, SQ], FR, name="wo_sb")
        mbq_sb = cpool.tile([128, NSKT + 2], FP, name="mbq_sb")
        ones_sb = cpool.tile([1, 128], FR, name="ones_sb")
        qT_sb = cpool.tile([128, 2, SQ], FR, name="qT_sb")
        kT_sb = cpool.tile([128, 2, skp], FR, name="kT_sb")
        vext_sb = cpool.tile([128, NSKT, 65 * HPC], FR, name="vext_sb")
        ctxT_sb = cpool.tile([128, 2, SQ], FR, name="ctxT_sb")
        o0_sb = cpool.tile([128, 8, SQ], BF, name="o0_sb")

        vv = vext_sb[:, :, :].rearrange("p s (h c) -> p s h c", c=65)
        mb = mbq_sb[:, 0:NSKT]
        wq_sb = wqv_sb[:, :, 0:DH]
        wv_sb = wqv_sb[:, :, DH:2 * DH]

        nc.sync.dma_start(out=wk_sb[:], in_=wk_d.rearrange("(kc p) n -> p kc n", p=128))

        with tc.tile_pool(name="pa", bufs=1, space="PSUM") as pa, \
             tc.tile_pool(name="sexp", bufs=CFG["sexp"]) as sexp, \
             tc.tile_pool(name="sout", bufs=CFG["sout"]) as sout, \
             tc.tile_pool(name="sin", bufs=CFG["sin"]) as sin:

            # vext ones columns (written once, before any v data lands)
            ones_fp = sout.tile([128, NSKT], FP, tag="onesfp", name="ones_fp")
            nc.vector.memset(ones_fp[:], 1.0)
            with nc.allow_low_precision(reason="float32r has float32 storage"):
                nc.vector.tensor_copy(
                    vv[:, :, :, 64:65],
                    ones_fp[:, :, None, None].to_broadcast((128, NSKT, HPC, 1)))

            def scores(h, skt, ps_dst, lo, w=512):
                t, r0 = h // 2, 64 * (h % 2)
                nc.tensor.matmul(
                    ps_dst[:, 0:w],
                    lhsT=kT_sb[r0:r0 + 64, t, skt * 128:(skt + 1) * 128],
                    rhs=qT_sb[r0:r0 + 64, t, lo:lo + w],
                    start=True, stop=True)

            def ctx(h, skt, es_src, acc, lo, w=512):
                nc.tensor.matmul(
                    acc[:, lo:lo + w],
                    lhsT=vv[:, skt, h, :],
                    rhs=es_src[:, 0:w],
                    start=(skt == 0), stop=(skt == NSKT - 1))

            def normalize(h, acc, psum_pool, shape, tag):
                # per-sq-half chain so downstream consumers of the first half
                # (e.g. P3 tiles 0-3) unblock sooner
                t, r0 = h // 2, 64 * (h % 2)
                rec = sout.tile([1, SQ], FR, tag="rec", name="rec")
                bc_sb = sout.tile([64, SQ], FP, tag="bc", name="bc_sb")
                for lo in range(0, SQ, 512):
                    with nc.allow_low_precision(reason="float32r storage"):
                        nc.vector.reciprocal(rec[:, lo:lo + 512],
                                             acc[64:65, lo:lo + 512])
                    ps_bc = psum_pool.tile(shape, FP, tag=tag, name="ps_bc")
                    nc.tensor.matmul(ps_bc[:64, 0:512],
                                     lhsT=ones_sb[0:1, 0:64],
                                     rhs=rec[0:1, lo:lo + 512],
                                     start=True, stop=True)
                    nc.vector.tensor_copy(bc_sb[:, lo:lo + 512],
                                          ps_bc[:64, 0:512])
                    with nc.allow_low_precision(reason="float32r storage"):
                        nc.vector.tensor_tensor(
                            ctxT_sb[r0:r0 + 64, t, lo:lo + 512],
                            acc[0:64, lo:lo + 512], bc_sb[:, lo:lo + 512], MUL)

            # ---- P1: heads {0, 1}; projections spread as PE filler ----
            with tc.tile_pool(name="pb", bufs=1, space="PSUM") as pb, \
                 tc.tile_pool(name="pk", bufs=1, space="PSUM") as pk, \
                 tc.tile_pool(name="ps1", bufs=3, space="PSUM") as ps1:

                stage = {}

                def emit_k_dma(b):
                    off, w = blocks[b]
                    xin = sin.tile([128, NKC, 512], BF, tag="sin", name="xin")
                    nc.sync.dma_start(
                        out=xin[:, :, 0:w],
                        in_=kx_d.rearrange("(kc p) n -> p kc n", p=128)[:, :, off:off + w])
                    stage[b] = [xin, None]

                def emit_v_dma(b):
                    off, w = blocks[b]
                    vin = sin.tile([128, NKC, 512], BF, tag="sin", name="vin")
                    nc.sync.dma_start(
                        out=vin[:, :, 0:w],
                        in_=vx_d.rearrange("(kc p) n -> p kc n", p=128)[:, :, off:off + w])
                    stage[b][1] = vin

                def emit_block_dma(b):
                    emit_k_dma(b)
                    emit_v_dma(b)

                kstate = {}

                def kproj_half(xin, off, w, t, half):
                    if half == 0:
                        kstate[(off, t)] = pk.tile([128, 512], FP, tag="k",
                                                   name="ps_k")
                    ps = kstate[(off, t)]
                    for kc in range(half * 4, half * 4 + 4):
                        nc.tensor.matmul(
                            ps[:, 0:w],
                            lhsT=wk_sb[:, kc, t * 128:(t + 1) * 128],
                            rhs=xin[:, kc, 0:w],
                            start=(kc == 0), stop=(kc == NKC - 1))
                    if half == 1:
                        with nc.allow_low_precision(reason="float32r storage"):
                            nc.vector.tensor_scalar_add(
                                kT_sb[:, t, off:off + w], ps[:, 0:w], 0.0)

                def qproj(xin, off, w, t):
                    ps = pk.tile([128, 512], FP, tag="k", name="ps_q")
                    for kc in range(NKC):
                        nc.tensor.matmul(
                            ps[:, 0:w],
                            lhsT=wq_sb[:, kc, t * 128:(t + 1) * 128],
                            rhs=xin[:, kc, off:off + w],
                            start=(kc == 0), stop=(kc == NKC - 1))
                    with nc.allow_low_precision(reason="float32r storage"):
                        nc.vector.tensor_scalar_add(
                            qT_sb[:, t, off:off + w], ps[:, 0:w],
                            mbq_sb[:, NSKT + t:NSKT + t + 1])

                def vproj(vin, off, s):
                    skt = (off + s) // 128
                    ps_v = pk.tile([128, 512], FP, tag="k", name="ps_v")[:, 0:DH]
                    for kc in range(NKC):
                        nc.tensor.matmul(
                            ps_v[:, :],
                            lhsT=vin[:, kc, s:s + 128],
                            rhs=wv_sb[:, kc, :],
                            start=(kc == 0), stop=(kc == NKC - 1))
                    with nc.allow_low_precision(reason="float32r storage"):
                        nc.vector.tensor_copy(
                            vv[:, skt, :, 0:64],
                            ps_v[:, :].rearrange("p (h c) -> p h c", c=64))

                def proj_groups(b):
                    off, w = blocks[b]
                    xin, vin = stage[b]
                    gs = []
                    for half in range(2):
                        gs.append(lambda h=half: kproj_half(xin, off, w, 0, h))
                    for s in range(0, w, 128):
                        gs.append(lambda s=s: vproj(vin, off, s))
                    for half in range(2):
                        gs.append(lambda h=half: kproj_half(xin, off, w, 1, h))
                    return gs

                # prologue DMA: K weights + block-0 keys first (shortest path
                # to PE work), then QV weights, Q input; block-0 values last
                # (first needed by the ctx of key-tile 0, well into P1)
                off0, w0 = blocks[0]
                xin_b0 = sin.tile([128, NKC, 512], BF, tag="sin", name="xin")
                kre = kx_d.rearrange("(kc p) n -> p kc n", p=128)
                nc.sync.dma_start(out=xin_b0[:, 0:4, 0:w0], in_=kre[:, 0:4, 0:w0])
                nc.sync.dma_start(out=xin_b0[:, 4:8, 0:w0], in_=kre[:, 4:8, 0:w0])
                stage[0] = [xin_b0, None]
                nc.sync.dma_start(out=wqv_sb[:],
                                  in_=wqv_d.rearrange("(kc p) n -> p kc n", p=128))
                nc.sync.dma_start(out=mbq_sb[:], in_=mbq_d[:, :])
                qre = qT_d.rearrange("(kc p) n -> p kc n", p=128)
                nc.sync.dma_start(out=qx_sb[:, :, 0:512], in_=qre[:, :, 0:512])
                nc.sync.dma_start(out=qx_sb[:, :, 512:1024], in_=qre[:, :, 512:1024])
                emit_v_dma(0)

                # prologue PE: t=0 projections only -- heads 0,1 are both in
                # t-tile 0; all t=1 projection work rides the filler stream
                # and must only complete before P2a's first scores
                xin0, vin0 = stage[0]
                for half in range(2):
                    kproj_half(xin0, 0, 512, 0, half)
                for off, w in blocks_of(SQ):
                    qproj(qx_sb, off, w, 0)

                acc0 = pa.tile([65, SQ], FP, tag="acc", name="acc")
                acc1 = pb.tile([65, SQ], FP, tag="acc1", name="acc1")

                fillers = [lambda s=s: vproj(vin0, 0, s) for s in range(0, 512, 128)]
                fillers += [lambda off=off, w=w: qproj(qx_sb, off, w, 1)
                            for off, w in blocks_of(SQ)]
                fillers += [lambda h=h: kproj_half(xin0, 0, 512, 1, h)
                            for h in range(2)]
                p1_pend = []

                def fill():
                    if fillers:
                        fillers.pop(0)()

                for b, (off, w) in enumerate(blocks):
                    if b + 1 < len(blocks):
                        emit_block_dma(b + 1)
                        fillers.extend(proj_groups(b + 1))
                    if b == 1:
                        nc.sync.dma_start(out=ones_sb[:], in_=ones_d[:, :])
                        nc.sync.dma_start(out=idn_sb[:], in_=idn_d[:, :])
                        nc.sync.dma_start(
                            out=wo_sb[:],
                            in_=wo_d.rearrange("(t p) n -> p t n", p=128))
                    for s in range(0, w, 128):
                        skt = (off + s) // 128
                        for h in range(2):
                            for lo in (0, 512):
                                ps_s = ps1.tile([128, 512], FP, tag="s",
                                                name="ps_s")
                                scores(h, skt, ps_s, lo)
                                es = sexp.tile([128, 512], FR, tag="es5",
                                               name="es5")
                                nc.scalar.activation(
                                    es[:, :], ps_s[:, :], AF.Exp,
                                    bias=mb[:, skt:skt + 1], scale=SCALE)
                                p1_pend.append((h, skt, lo, es))
                                if (h, lo) != (0, 0):
                                    fill()
                                if len(p1_pend) > 2:
                                    ph, pskt, plo, pes = p1_pend.pop(0)
                                    ctx(ph, pskt, pes,
                                        acc0 if ph == 0 else acc1, plo)

                while fillers:
                    fillers.pop(0)()
                for ph, pskt, plo, pes in p1_pend:
                    ctx(ph, pskt, pes, acc0 if ph == 0 else acc1, plo)
                normalize(0, acc0, ps1, [128, 512], "s")
                normalize(1, acc1, ps1, [128, 512], "s")

            # ---- P2a: head 2 + t=0 half of the output projection ----
            with tc.tile_pool(name="ps2", bufs=2, space="PSUM") as ps2, \
                 tc.tile_pool(name="po0", bufs=2, space="PSUM") as po0:
                acc2 = pa.tile([65, SQ], FP, tag="acc", name="acc")

                def out_t0(sq, lo):
                    ps_o = po0.tile([128, 512], FP, tag="o0", name="ps_o0")
                    nc.tensor.matmul(
                        ps_o[:, :],
                        lhsT=ctxT_sb[:, 0, sq * 128:(sq + 1) * 128],
                        rhs=wo_sb[:, 0, lo:lo + 512],
                        start=True, stop=True)
                    with nc.allow_low_precision(reason="bf16 staging"):
                        nc.vector.tensor_scalar_add(o0_sb[:, sq, lo:lo + 512],
                                                    ps_o[:, :], 0.0)

                pend = []
                ohalf = 0
                for skt in range(NSKT):
                    ps_s = ps2.tile([128, SQ], FP, tag="s2", name="ps_p2")
                    scores(2, skt, ps_s[:, 0:512], 0)
                    scores(2, skt, ps_s[:, 512:1024], 512)
                    es = sexp.tile([128, SQ], FR, tag="es", name="es")
                    nc.scalar.activation(
                        es[:, :], ps_s[:, :], AF.Exp,
                        bias=mb[:, skt:skt + 1], scale=SCALE)
                    pend.append((skt, es))
                    if ohalf < 16:
                        out_t0(ohalf // 2, 512 * (ohalf % 2))
                        ohalf += 1
                    if len(pend) > 2:
                        pskt, pes = pend.pop(0)
                        ctx(2, pskt, pes[:, 0:512], acc2, 0)
                        ctx(2, pskt, pes[:, 512:1024], acc2, 512)
                while ohalf < 16:
                    out_t0(ohalf // 2, 512 * (ohalf % 2))
                    ohalf += 1
                for pskt, pes in pend:
                    ctx(2, pskt, pes[:, 0:512], acc2, 0)
                    ctx(2, pskt, pes[:, 512:1024], acc2, 512)

                # ---- P2b: head 3; head-2 normalize folded in ----
                acc3 = pa.tile([65, SQ], FP, tag="acc", name="acc")
                pend = []
                for skt in range(NSKT):
                    ps_s = ps2.tile([128, SQ], FP, tag="s2", name="ps_p2")
                    scores(3, skt, ps_s[:, 0:512], 0)
                    scores(3, skt, ps_s[:, 512:1024], 512)
                    es = sexp.tile([128, SQ], FR, tag="es", name="es")
                    nc.scalar.activation(
                        es[:, :], ps_s[:, :], AF.Exp,
                        bias=mb[:, skt:skt + 1], scale=SCALE)
                    pend.append((skt, es))
                    if skt == 1:
                        normalize(2, acc2, po0, [128, 512], "o0")
                    if len(pend) > 2:
                        pskt, pes = pend.pop(0)
                        ctx(3, pskt, pes[:, 0:512], acc3, 0)
                        ctx(3, pskt, pes[:, 512:1024], acc3, 512)
                for pskt, pes in pend:
                    ctx(3, pskt, pes[:, 0:512], acc3, 0)
                    ctx(3, pskt, pes[:, 512:1024], acc3, 512)
                normalize(3, acc3, po0, [128, 512], "o0")

                # ---- P3: out = staged t0 (identity-matmul, independent of
                # the head-3 normalize) + t1 matmul; out staged in one SBUF
                # buffer and shipped as two large DMAs ----
                ob_sb = cpool.tile([128, 8, SQ], BF, name="ob_sb")
                for sq in range(SQ // 128):
                    ps_o = ps2.tile([128, SQ], FP, tag="s2", name="ps_o")
                    for lo in range(0, SQ, 512):
                        nc.tensor.matmul(
                            ps_o[:, lo:lo + 512],
                            lhsT=idn_sb[:, :],
                            rhs=o0_sb[:, sq, lo:lo + 512],
                            start=True, stop=False)
                        nc.tensor.matmul(
                            ps_o[:, lo:lo + 512],
                            lhsT=ctxT_sb[:, 1, sq * 128:(sq + 1) * 128],
                            rhs=wo_sb[:, 1, lo:lo + 512],
                            start=False, stop=True)
                    if sq % 2 == 0:
                        nc.scalar.activation(ob_sb[:, sq, :], ps_o[:], AF.Copy)
                    else:
                        with nc.allow_low_precision(reason="bf16 output"):
                            nc.vector.tensor_scalar_add(ob_sb[:, sq, :], ps_o[:], 0.0)
                    if sq == 3:
                        nc.sync.dma_start(
                            out=out_d.rearrange("(t p) n -> p t n", p=128)[:, 0:4, :],
                            in_=ob_sb[:, 0:4, :])
                nc.sync.dma_start(
                    out=out_d.rearrange("(t p) n -> p t n", p=128)[:, 4:8, :],
                    in_=ob_sb[:, 4:8, :])

        cpool_cm.__exit__(None, None, None)

    nc.compile()
    return nc


def get_nc(skp=SK):
    key = ("nc", skp)
    if key not in _CACHE:
        _CACHE[key] = _build(skp)
    return _CACHE[key]


def make_in_maps(query, key, value, key_mask, Wq, bq, Wk, bk, Wv, bv, Wo, bo):
    f32 = lambda x: np.asarray(x, dtype=np.float32)
    bf16 = lambda x: np.ascontiguousarray(np.asarray(x, np.float32).astype(ml_dtypes.bfloat16))
    query, key, value = f32(query), f32(key), f32(value)
    Wq, bq, Wk = f32(Wq), f32(bq), f32(Wk)
    Wv, bv, Wo, bo = f32(Wv), f32(bv), f32(Wo), f32(bo)
    key_mask = np.asarray(key_mask)

    # compact unmasked keys; pad to a common multiple of 128
    keep = [np.nonzero(key_mask[b] != 0)[0] for b in range(B)]
    skp = max(512, int(-(-max(len(k) for k in keep) // 128) * 128))
    skp = min(skp, SK)
    nskt = skp // 128

    # bk dropped (softmax shift-invariance); bv folded into the host-side
    # output bias:  out += (bv @ Wo);  bo handled on host too.
    bo_eff = bo + bv @ Wo

    qT, kxT, vxT, mb = [], [], [], []
    for b in range(B):
        n = len(keep[b])
        kc = np.zeros((skp, IN), np.float32)
        vc = np.zeros((skp, IN), np.float32)
        kc[:n] = key[b][keep[b]]
        vc[:n] = value[b][keep[b]]
        mbias = np.full(skp, -1e9, np.float32)
        mbias[:n] = 0.0
        qT.append(bf16(query[b].T))
        kxT.append(bf16(kc.T))
        vxT.append(bf16(vc.T))
        mb.append(np.ascontiguousarray(mbias.reshape(nskt, 128).T))

    in_maps = []
    for c in range(NCORES):
        b, g = c // 4, c % 4
        S = slice(DH * g, DH * (g + 1))
        mbq = np.concatenate(
            [mb[b], bq[S][0:128][:, None], bq[S][128:256][:, None]], axis=1)
        in_maps.append({
            "qT": qT[b], "kx": kxT[b], "vx": vxT[b],
            "wk": bf16(Wk[:, S]),
            "wqv": bf16(np.concatenate([Wq[:, S], Wv[:, S]], axis=1)),
            "wo": np.ascontiguousarray(Wo[S, :]),
            "mbq": np.ascontiguousarray(mbq),
            "ones": np.ones((1, 128), np.float32),
            "idn": np.eye(128, dtype=ml_dtypes.bfloat16),
        })
    return in_maps, skp, bo_eff


def run(in_maps, skp=SK, trace=False):
    from concourse.bass_utils import run_bass_kernel_spmd
    nc = get_nc(skp)
    res = run_bass_kernel_spmd(nc, in_maps, list(range(NCORES)), trace=trace)
    _CACHE["last_results"] = res
    return res


def kernel(query, key, value, key_mask, Wq, bq, Wk, bk, Wv, bv, Wo, bo):
    in_maps, skp, bo_eff = make_in_maps(query, key, value, key_mask,
                                        Wq, bq, Wk, bk, Wv, bv, Wo, bo)
    res = run(in_maps, skp)
    out = np.zeros((B, SQ, SQ), np.float32)
    for c in range(NCORES):
        out[c // 4] += np.asarray(res.results[c]["out"], np.float32)
    out += bo_eff[None, None, :]
    return out


# revision 28
# speedup vs baseline: 1.0360x; 1.0360x over previous
"""Multi-head cross-attention Trainium2 kernel.

Full-input contract: kernel(**inputs) takes the complete tensors and returns
the complete output. Internally shards over 8 NeuronCores as
(batch x head-group): core c handles batch c//4 and heads [4*(c%4), 4*(c%4)+4).
Each core computes its partial output  ctx_g @ Wo_g  for its batch; the host
sums the 4 head-group partials per batch and adds bo.

Masked keys (key_mask == 0) contribute exactly zero probability, so the host
compacts key/value to the unmasked rows (padded up to a multiple of 128 with
-1e9 score bias), which shrinks the K/V projections and the whole attention
core proportionally.

Numerics: HBM-resident activations and projection weights are bf16 (halves
the DMA-bound), everything on-chip is fp32 (fp32r for matmul operands, which
streams at full rate for moving dims >= 256). Exact algebraic cuts:
  - bk is dropped entirely: a per-query constant added to every key's score
    is softmax-invariant.
  - bv is folded into bo on the host: probs sum to 1, so ctx = P@(v + bv) =
    P@v + bv, hence out += bv @ Wo, absorbed into bo.
  - bq is applied on-device in the Q projection.

Engines execute their queues in emission order, so the emission is software
pipelined (PSUM is the scarce resource - 8 banks):
  P1  prologue: block-0 K projection + Q projection.  Then heads 0,1
      attention per 128-key tile; the remaining K/V projection matmul groups
      (value-chunk-stationary V proj lands directly in [sk, d] layout - no PE
      transpose) are spread between dependent score/exp/ctx ops as PE filler
      to hide the ~800ns PSUM-drain+semaphore latency around each exp.
  P2a head 2 attention (one 1024-wide exp per key tile, ctx lagging 2 units
      so the pipeline is ACT-rate-limited), interleaved with the t=0 half of
      the output projection (heads 0,1 context ready) into 2 spare PSUM
      banks, staged to SBUF in fp32.
  P2b head 3 attention, with head-2 normalization folded in.
  P3  out = t=1 half + staged t=0 half, written back as bf16; host upcasts
      and reduces.
"""

import numpy as np
import ml_dtypes

B, SQ, SK, IN = 2, 1024, 4096, 1024
H_TOT, D, HPC = 16, 64, 4
DH = HPC * D  # 256, per-core head-dim slice
NCORES = 8

_CACHE = {}

# scheduling knobs
CFG = {"sin": 4, "sexp": 4, "sout": 3}


def _build(skp):
    import concourse.tile as tile
    from concourse import bacc, mybir

    FP = mybir.dt.float32
    FR = mybir.dt.float32r
    BF = mybir.dt.bfloat16
    AF = mybir.ActivationFunctionType
    MUL = mybir.AluOpType.mult
    ADD = mybir.AluOpType.add

    nc = bacc.Bacc("TRN2", target_bir_lowering=False, debug=False)

    NSKT = skp // 128          # sk tiles of 128
    NKC = IN // 128            # 8 contraction chunks
    SCALE = 1.0 / float(np.sqrt(D))

    qT_d = nc.dram_tensor("qT", [IN, SQ], BF, kind="ExternalInput").ap()
    kx_d = nc.dram_tensor("kx", [IN, skp], BF, kind="ExternalInput").ap()
    vx_d = nc.dram_tensor("vx", [IN, skp], BF, kind="ExternalInput").ap()
    wk_d = nc.dram_tensor("wk", [IN, DH], BF, kind="ExternalInput").ap()
    wqv_d = nc.dram_tensor("wqv", [IN, 2 * DH], BF, kind="ExternalInput").ap()
    wo_d = nc.dram_tensor("wo", [DH, SQ], FR, kind="ExternalInput").ap()
    # packed per-partition constants: cols [0:NSKT) mask bias, [NSKT:NSKT+2) bq
    mbq_d = nc.dram_tensor("mbq", [128, NSKT + 2], FP, kind="ExternalInput").ap()
    ones_d = nc.dram_tensor("ones", [1, 128], FR, kind="ExternalInput").ap()
    idn_d = nc.dram_tensor("idn", [128, 128], BF, kind="ExternalInput").ap()
    out_d = nc.dram_tensor("out", [SQ, SQ], BF, kind="ExternalOutput").ap()

    def blocks_of(width, step=512):
        out, off = [], 0
        while off < width:
            w = min(step, width - off)
            out.append((off, w))
            off += w
        return out

    blocks = blocks_of(skp)

    with tile.TileContext(nc) as tc:
        # ---- resident tensors (one bufs=1 pool; distinct names = own slots) ----
        cpool_cm = tc.tile_pool(name="const", bufs=1)
        cpool = cpool_cm.__enter__()
        wk_sb = cpool.tile([128, NKC, DH], BF, name="wk_sb")
        wqv_sb = cpool.tile([128, NKC, 2 * DH], BF, name="wqv_sb")
        qx_sb = cpool.tile([128, NKC, SQ], BF, name="qx_sb")
        idn_sb = cpool.tile([128, 128], BF, name="idn_sb")
        wo_sb = cpool.tile([128, 2, SQ], FR, name="wo_sb")
        mbq_sb = cpool.tile([128, NSKT + 2], FP, name="mbq_sb")
        ones_sb = cpool.tile([1, 128], FR, name="ones_sb")
        qT_sb = cpool.tile([128, 2, SQ], FR, name="qT_sb")
        kT_sb = cpool.tile([128, 2, skp], FR, name="kT_sb")
        vext_sb = cpool.tile([128, NSKT, 65 * HPC], FR, name="vext_sb")
        ctxT_sb = cpool.tile([128, 2, SQ], FR, name="ctxT_sb")
        o0_sb = cpool.tile([128, 8, SQ], BF, name="o0_sb")

        vv = vext_sb[:, :, :].rearrange("p s (h c) -> p s h c", c=65)
        mb = mbq_sb[:, 0:NSKT]
        wq_sb = wqv_sb[:, :, 0:DH]
        wv_sb = wqv_sb[:, :, DH:2 * DH]

        nc.sync.dma_start(out=wk_sb[:], in_=wk_d.rearrange("(kc p) n -> p kc n", p=128))

        with tc.tile_pool(name="pa", bufs=1, space="PSUM") as pa, \
             tc.tile_pool(name="sexp", bufs=CFG["sexp"]) as sexp, \
             tc.tile_pool(name="sout", bufs=CFG["sout"]) as sout, \
             tc.tile_pool(name="sin", bufs=CFG["sin"]) as sin:

            # vext ones columns (written once, before any v data lands)
            ones_fp = sout.tile([128, NSKT], FP, tag="onesfp", name="ones_fp")
            nc.vector.memset(ones_fp[:], 1.0)
            with nc.allow_low_precision(reason="float32r has float32 storage"):
                nc.vector.tensor_copy(
                    vv[:, :, :, 64:65],
                    ones_fp[:, :, None, None].to_broadcast((128, NSKT, HPC, 1)))

            def scores(h, skt, ps_dst, lo, w=512):
                t, r0 = h // 2, 64 * (h % 2)
                nc.tensor.matmul(
                    ps_dst[:, 0:w],
                    lhsT=kT_sb[r0:r0 + 64, t, skt * 128:(skt + 1) * 128],
                    rhs=qT_sb[r0:r0 + 64, t, lo:lo + w],
                    start=True, stop=True)

            def ctx(h, skt, es_src, acc, lo, w=512):
                nc.tensor.matmul(
                    acc[:, lo:lo + w],
                    lhsT=vv[:, skt, h, :],
                    rhs=es_src[:, 0:w],
                    start=(skt == 0), stop=(skt == NSKT - 1))

            def normalize(h, acc, psum_pool, shape, tag):
                # per-sq-half chain so downstream consumers of the first half
                # (e.g. P3 tiles 0-3) unblock sooner
                t, r0 = h // 2, 64 * (h % 2)
                rec = sout.tile([1, SQ], FR, tag="rec", name="rec")
                bc_sb = sout.tile([64, SQ], FP, tag="bc", name="bc_sb")
                for lo in range(0, SQ, 512):
                    with nc.allow_low_precision(reason="float32r storage"):
                        nc.vector.reciprocal(rec[:, lo:lo + 512],
                                             acc[64:65, lo:lo + 512])
                    ps_bc = psum_pool.tile(shape, FP, tag=tag, name="ps_bc")
                    nc.tensor.matmul(ps_bc[:64, 0:512],
                                     lhsT=ones_sb[0:1, 0:64],
                                     rhs=rec[0:1, lo:lo + 512],
                                     start=True, stop=True)
                    nc.vector.tensor_copy(bc_sb[:, lo:lo + 512],
                                          ps_bc[:64, 0:512])
                    with nc.allow_low_precision(reason="float32r storage"):
                        nc.vector.tensor_tensor(
                            ctxT_sb[r0:r0 + 64, t, lo:lo + 512],
                            acc[0:64, lo:lo + 512], bc_sb[:, lo:lo + 512], MUL)

            # ---- P1: heads {0, 1}; projections spread as PE filler ----
            with tc.tile_pool(name="pb", bufs=1, space="PSUM") as pb, \
                 tc.tile_pool(name="pk", bufs=1, space="PSUM") as pk, \
                 tc.tile_pool(name="ps1", bufs=3, space="PSUM") as ps1:

                stage = {}

                def emit_k_dma(b):
                    off, w = blocks[b]
                    xin = sin.tile([128, NKC, 512], BF, tag="sin", name="xin")
                    nc.sync.dma_start(
                        out=xin[:, :, 0:w],
                        in_=kx_d.rearrange("(kc p) n -> p kc n", p=128)[:, :, off:off + w])
                    stage[b] = [xin, None]

                def emit_v_dma(b):
                    off, w = blocks[b]
                    vin = sin.tile([128, NKC, 512], BF, tag="sin", name="vin")
                    nc.sync.dma_start(
                        out=vin[:, :, 0:w],
                        in_=vx_d.rearrange("(kc p) n -> p kc n", p=128)[:, :, off:off + w])
                    stage[b][1] = vin

                def emit_block_dma(b):
                    emit_k_dma(b)
                    emit_v_dma(b)

                kstate = {}

                def kproj_half(xin, off, w, t, half):
                    if half == 0:
                        kstate[(off, t)] = pk.tile([128, 512], FP, tag="k",
                                                   name="ps_k")
                    ps = kstate[(off, t)]
                    for kc in range(half * 4, half * 4 + 4):
                        nc.tensor.matmul(
                            ps[:, 0:w],
                            lhsT=wk_sb[:, kc, t * 128:(t + 1) * 128],
                            rhs=xin[:, kc, 0:w],
                            start=(kc == 0), stop=(kc == NKC - 1))
                    if half == 1:
                        with nc.allow_low_precision(reason="float32r storage"):
                            nc.vector.tensor_scalar_add(
                                kT_sb[:, t, off:off + w], ps[:, 0:w], 0.0)

                def qproj(xin, off, w, t):
                    ps = pk.tile([128, 512], FP, tag="k", name="ps_q")
                    for kc in range(NKC):
                        nc.tensor.matmul(
                            ps[:, 0:w],
                            lhsT=wq_sb[:, kc, t * 128:(t + 1) * 128],
                            rhs=xin[:, kc, off:off + w],
                            start=(kc == 0), stop=(kc == NKC - 1))
                    with nc.allow_low_precision(reason="float32r storage"):
                        nc.vector.tensor_scalar_add(
                            qT_sb[:, t, off:off + w], ps[:, 0:w],
                            mbq_sb[:, NSKT + t:NSKT + t + 1])

                def vproj(vin, off, s):
                    skt = (off + s) // 128
                    ps_v = pk.tile([128, 512], FP, tag="k", name="ps_v")[:, 0:DH]
                    for kc in range(NKC):
                        nc.tensor.matmul(
                            ps_v[:, :],
                            lhsT=vin[:, kc, s:s + 128],
                            rhs=wv_sb[:, kc, :],
                            start=(kc == 0), stop=(kc == NKC - 1))
                    with nc.allow_low_precision(reason="float32r storage"):
                        nc.vector.tensor_copy(
                            vv[:, skt, :, 0:64],
                            ps_v[:, :].rearrange("p (h c) -> p h c", c=64))

                def proj_groups(b):
                    off, w = blocks[b]
                    xin, vin = stage[b]
                    gs = []
                    for half in range(2):
                        gs.append(lambda h=half: kproj_half(xin, off, w, 0, h))
                    for s in range(0, w, 128):
                        gs.append(lambda s=s: vproj(vin, off, s))
                    for half in range(2):
                        gs.append(lambda h=half: kproj_half(xin, off, w, 1, h))
                    return gs

                # prologue DMA: K weights + block-0 keys first (shortest path
                # to PE work), then QV weights, Q input; block-0 values last
                # (first needed by the ctx of key-tile 0, well into P1)
                emit_k_dma(0)
                nc.sync.dma_start(out=wqv_sb[:],
                                  in_=wqv_d.rearrange("(kc p) n -> p kc n", p=128))
                nc.sync.dma_start(out=mbq_sb[:], in_=mbq_d[:, :])
                qre = qT_d.rearrange("(kc p) n -> p kc n", p=128)
                nc.sync.dma_start(out=qx_sb[:, :, 0:512], in_=qre[:, :, 0:512])
                nc.sync.dma_start(out=qx_sb[:, :, 512:1024], in_=qre[:, :, 512:1024])
                emit_v_dma(0)

                # prologue PE: t=0 projections only -- heads 0,1 are both in
                # t-tile 0; all t=1 projection work rides the filler stream
                # and must only complete before P2a's first scores
                xin0, vin0 = stage[0]
                for half in range(2):
                    kproj_half(xin0, 0, 512, 0, half)
                for off, w in blocks_of(SQ):
                    qproj(qx_sb, off, w, 0)

                acc0 = pa.tile([65, SQ], FP, tag="acc", name="acc")
                acc1 = pb.tile([65, SQ], FP, tag="acc1", name="acc1")

                fillers = [lambda s=s: vproj(vin0, 0, s) for s in range(0, 512, 128)]
                fillers += [lambda off=off, w=w: qproj(qx_sb, off, w, 1)
                            for off, w in blocks_of(SQ)]
                fillers += [lambda h=h: kproj_half(xin0, 0, 512, 1, h)
                            for h in range(2)]
                p1_pend = []

                def fill():
                    if fillers:
                        fillers.pop(0)()

                for b, (off, w) in enumerate(blocks):
                    if b + 1 < len(blocks):
                        emit_block_dma(b + 1)
                        fillers.extend(proj_groups(b + 1))
                    if b == 1:
                        nc.sync.dma_start(out=ones_sb[:], in_=ones_d[:, :])
                        nc.sync.dma_start(out=idn_sb[:], in_=idn_d[:, :])
                        nc.sync.dma_start(
                            out=wo_sb[:],
                            in_=wo_d.rearrange("(t p) n -> p t n", p=128))
                    for s in range(0, w, 128):
                        skt = (off + s) // 128
                        for h in range(2):
                            for lo in (0, 512):
                                ps_s = ps1.tile([128, 512], FP, tag="s",
                                                name="ps_s")
                                scores(h, skt, ps_s, lo)
                                es = sexp.tile([128, 512], FR, tag="es5",
                                               name="es5")
                                nc.scalar.activation(
                                    es[:, :], ps_s[:, :], AF.Exp,
                                    bias=mb[:, skt:skt + 1], scale=SCALE)
                                p1_pend.append((h, skt, lo, es))
                                if (h, lo) != (0, 0):
                                    fill()
                                if len(p1_pend) > 2:
                                    ph, pskt, plo, pes = p1_pend.pop(0)
                                    ctx(ph, pskt, pes,
                                        acc0 if ph == 0 else acc1, plo)

                while fillers:
                    fillers.pop(0)()
                for ph, pskt, plo, pes in p1_pend:
                    ctx(ph, pskt, pes, acc0 if ph == 0 else acc1, plo)
                normalize(0, acc0, ps1, [128, 512], "s")
                normalize(1, acc1, ps1, [128, 512], "s")

            # ---- P2a: head 2 + t=0 half of the output projection ----
            with tc.tile_pool(name="ps2", bufs=2, space="PSUM") as ps2, \
                 tc.tile_pool(name="po0", bufs=2, space="PSUM") as po0:
                acc2 = pa.tile([65, SQ], FP, tag="acc", name="acc")

                def out_t0(sq, lo):
                    ps_o = po0.tile([128, 512], FP, tag="o0", name="ps_o0")
                    nc.tensor.matmul(
                        ps_o[:, :],
                        lhsT=ctxT_sb[:, 0, sq * 128:(sq + 1) * 128],
                        rhs=wo_sb[:, 0, lo:lo + 512],
                        start=True, stop=True)
                    with nc.allow_low_precision(reason="bf16 staging"):
                        nc.vector.tensor_scalar_add(o0_sb[:, sq, lo:lo + 512],
                                                    ps_o[:, :], 0.0)

                pend = []
                ohalf = 0
                for skt in range(NSKT):
                    ps_s = ps2.tile([128, SQ], FP, tag="s2", name="ps_p2")
                    scores(2, skt, ps_s[:, 0:512], 0)
                    scores(2, skt, ps_s[:, 512:1024], 512)
                    es = sexp.tile([128, SQ], FR, tag="es", name="es")
                    nc.scalar.activation(
                        es[:, :], ps_s[:, :], AF.Exp,
                        bias=mb[:, skt:skt + 1], scale=SCALE)
                    pend.append((skt, es))
                    if ohalf < 16:
                        out_t0(ohalf // 2, 512 * (ohalf % 2))
                        ohalf += 1
                    if len(pend) > 2:
                        pskt, pes = pend.pop(0)
                        ctx(2, pskt, pes[:, 0:512], acc2, 0)
                        ctx(2, pskt, pes[:, 512:1024], acc2, 512)
                while ohalf < 16:
                    out_t0(ohalf // 2, 512 * (ohalf % 2))
                    ohalf += 1
                for pskt, pes in pend:
                    ctx(2, pskt, pes[:, 0:512], acc2, 0)
                    ctx(2, pskt, pes[:, 512:1024], acc2, 512)

                # ---- P2b: head 3; head-2 normalize folded in ----
                acc3 = pa.tile([65, SQ], FP, tag="acc", name="acc")
                pend = []
                for skt in range(NSKT):
                    ps_s = ps2.tile([128, SQ], FP, tag="s2", name="ps_p2")
                    scores(3, skt, ps_s[:, 0:512], 0)
                    scores(3, skt, ps_s[:, 512:1024], 512)
                    es = sexp.tile([128, SQ], FR, tag="es", name="es")
                    nc.scalar.activation(
                        es[:, :], ps_s[:, :], AF.Exp,
                        bias=mb[:, skt:skt + 1], scale=SCALE)
                    pend.append((skt, es))
                    if skt == 1:
                        normalize(2, acc2, po0, [128, 512], "o0")
                    if len(pend) > 2:
                        pskt, pes = pend.pop(0)
                        ctx(3, pskt, pes[:, 0:512], acc3, 0)
                        ctx(3, pskt, pes[:, 512:1024], acc3, 512)
                for pskt, pes in pend:
                    ctx(3, pskt, pes[:, 0:512], acc3, 0)
                    ctx(3, pskt, pes[:, 512:1024], acc3, 512)
                normalize(3, acc3, po0, [128, 512], "o0")

                # ---- P3: out = staged t0 (identity-matmul, independent of
                # the head-3 normalize) + t1 matmul; out staged in one SBUF
                # buffer and shipped as two large DMAs ----
                ob_sb = cpool.tile([128, 8, SQ], BF, name="ob_sb")
                for sq in range(SQ // 128):
                    ps_o = ps2.tile([128, SQ], FP, tag="s2", name="ps_o")
                    for lo in range(0, SQ, 512):
                        nc.tensor.matmul(
                            ps_o[:, lo:lo + 512],
                            lhsT=idn_sb[:, :],
                            rhs=o0_sb[:, sq, lo:lo + 512],
                            start=True, stop=False)
                        nc.tensor.matmul(
                            ps_o[:, lo:lo + 512],
                            lhsT=ctxT_sb[:, 1, sq * 128:(sq + 1) * 128],
                            rhs=wo_sb[:, 1, lo:lo + 512],
                            start=False, stop=True)
                    if sq % 2 == 0:
                        nc.scalar.activation(ob_sb[:, sq, :], ps_o[:], AF.Copy)
                    else:
                        with nc.allow_low_precision(reason="bf16 output"):
                            nc.vector.tensor_scalar_add(ob_sb[:, sq, :], ps_o[:], 0.0)
                    if sq == 3:
                        nc.sync.dma_start(
                            out=out_d.rearrange("(t p) n -> p t n", p=128)[:, 0:4, :],
                            in_=ob_sb[:, 0:4, :])
                nc.sync.dma_start(
                    out=out_d.rearrange("(t p) n -> p t n", p=128)[:, 4:8, :],
                    in_=ob_sb[:, 4:8, :])

        cpool_cm.__exit__(None, None, None)

    nc.compile()
    return nc


def get_nc(skp=SK):
    key = ("nc", skp)
    if key not in _CACHE:
        _CACHE[key] = _build(skp)
    return _CACHE[key]


def make_in_maps(query, key, value, key_mask, Wq, bq, Wk, bk, Wv, bv, Wo, bo):
    f32 = lambda x: np.asarray(x, dtype=np.float32)
    bf16 = lambda x: np.ascontiguousarray(np.asarray(x, np.float32).astype(ml_dtypes.bfloat16))
    query, key, value = f32(query), f32(key), f32(value)
    Wq, bq, Wk = f32(Wq), f32(bq), f32(Wk)
    Wv, bv, Wo, bo = f32(Wv), f32(bv), f32(Wo), f32(bo)
    key_mask = np.asarray(key_mask)

    # compact unmasked keys; pad to a common multiple of 128
    keep = [np.nonzero(key_mask[b] != 0)[0] for b in range(B)]
    skp = max(512, int(-(-max(len(k) for k in keep) // 128) * 128))
    skp = min(skp, SK)
    nskt = skp // 128

    # bk dropped (softmax shift-invariance); bv folded into the host-side
    # output bias:  out += (bv @ Wo);  bo handled on host too.
    bo_eff = bo + bv @ Wo

    qT, kxT, vxT, mb = [], [], [], []
    for b in range(B):
        n = len(keep[b])
        kc = np.zeros((skp, IN), np.float32)
        vc = np.zeros((skp, IN), np.float32)
        kc[:n] = key[b][keep[b]]
        vc[:n] = value[b][keep[b]]
        mbias = np.full(skp, -1e9, np.float32)
        mbias[:n] = 0.0
        qT.append(bf16(query[b].T))
        kxT.append(bf16(kc.T))
        vxT.append(bf16(vc.T))
        mb.append(np.ascontiguousarray(mbias.reshape(nskt, 128).T))

    in_maps = []
    for c in range(NCORES):
        b, g = c // 4, c % 4
        S = slice(DH * g, DH * (g + 1))
        mbq = np.concatenate(
            [mb[b], bq[S][0:128][:, None], bq[S][128:256][:, None]], axis=1)
        in_maps.append({
            "qT": qT[b], "kx": kxT[b], "vx": vxT[b],
            "wk": bf16(Wk[:, S]),
            "wqv": bf16(np.concatenate([Wq[:, S], Wv[:, S]], axis=1)),
            "wo": np.ascontiguousarray(Wo[S, :]),
            "mbq": np.ascontiguousarray(mbq),
            "ones": np.ones((1, 128), np.float32),
            "idn": np.eye(128, dtype=ml_dtypes.bfloat16),
        })
    return in_maps, skp, bo_eff


def run(in_maps, skp=SK, trace=False):
    from concourse.bass_utils import run_bass_kernel_spmd
    nc = get_nc(skp)
    res = run_bass_kernel_spmd(nc, in_maps, list(range(NCORES)), trace=trace)
    _CACHE["last_results"] = res
    return res


def kernel(query, key, value, key_mask, Wq, bq, Wk, bk, Wv, bv, Wo, bo):
    in_maps, skp, bo_eff = make_in_maps(query, key, value, key_mask,
                                        Wq, bq, Wk, bk, Wv, bv, Wo, bo)
    res = run(in_maps, skp)
    out = np.zeros((B, SQ, SQ), np.float32)
    for c in range(NCORES):
        out[c // 4] += np.asarray(res.results[c]["out"], np.float32)
    out += bo_eff[None, None, :]
    return out


# revision 29
# speedup vs baseline: 1.0429x; 1.0066x over previous
"""Multi-head cross-attention Trainium2 kernel.

Full-input contract: kernel(**inputs) takes the complete tensors and returns
the complete output. Internally shards over 8 NeuronCores as
(batch x head-group): core c handles batch c//4 and heads [4*(c%4), 4*(c%4)+4).
Each core computes its partial output  ctx_g @ Wo_g  for its batch; the host
sums the 4 head-group partials per batch and adds bo.

Masked keys (key_mask == 0) contribute exactly zero probability, so the host
compacts key/value to the unmasked rows (padded up to a multiple of 128 with
-1e9 score bias), which shrinks the K/V projections and the whole attention
core proportionally.

Numerics: HBM-resident activations and projection weights are bf16 (halves
the DMA-bound), everything on-chip is fp32 (fp32r for matmul operands, which
streams at full rate for moving dims >= 256). Exact algebraic cuts:
  - bk is dropped entirely: a per-query constant added to every key's score
    is softmax-invariant.
  - bv is folded into bo on the host: probs sum to 1, so ctx = P@(v + bv) =
    P@v + bv, hence out += bv @ Wo, absorbed into bo.
  - bq is applied on-device in the Q projection.

Engines execute their queues in emission order, so the emission is software
pipelined (PSUM is the scarce resource - 8 banks):
  P1  prologue: block-0 K projection + Q projection.  Then heads 0,1
      attention per 128-key tile; the remaining K/V projection matmul groups
      (value-chunk-stationary V proj lands directly in [sk, d] layout - no PE
      transpose) are spread between dependent score/exp/ctx ops as PE filler
      to hide the ~800ns PSUM-drain+semaphore latency around each exp.
  P2a head 2 attention (one 1024-wide exp per key tile, ctx lagging 2 units
      so the pipeline is ACT-rate-limited), interleaved with the t=0 half of
      the output projection (heads 0,1 context ready) into 2 spare PSUM
      banks, staged to SBUF in fp32.
  P2b head 3 attention, with head-2 normalization folded in.
  P3  out = t=1 half + staged t=0 half, written back as bf16; host upcasts
      and reduces.
"""

import numpy as np
import ml_dtypes

B, SQ, SK, IN = 2, 1024, 4096, 1024
H_TOT, D, HPC = 16, 64, 4
DH = HPC * D  # 256, per-core head-dim slice
NCORES = 8

_CACHE = {}

# scheduling knobs
CFG = {"sin": 4, "sexp": 6, "sout": 3}


def _build(skp):
    import concourse.tile as tile
    from concourse import bacc, mybir

    FP = mybir.dt.float32
    FR = mybir.dt.float32r
    BF = mybir.dt.bfloat16
    AF = mybir.ActivationFunctionType
    MUL = mybir.AluOpType.mult
    ADD = mybir.AluOpType.add

    nc = bacc.Bacc("TRN2", target_bir_lowering=False, debug=False)

    NSKT = skp // 128          # sk tiles of 128
    NKC = IN // 128            # 8 contraction chunks
    SCALE = 1.0 / float(np.sqrt(D))

    qT_d = nc.dram_tensor("qT", [IN, SQ], BF, kind="ExternalInput").ap()
    kx_d = nc.dram_tensor("kx", [IN, skp], BF, kind="ExternalInput").ap()
    vx_d = nc.dram_tensor("vx", [IN, skp], BF, kind="ExternalInput").ap()
    wk_d = nc.dram_tensor("wk", [IN, DH], BF, kind="ExternalInput").ap()
    wqv_d = nc.dram_tensor("wqv", [IN, 2 * DH], BF, kind="ExternalInput").ap()
    wo_d = nc.dram_tensor("wo", [DH, SQ], FR, kind="ExternalInput").ap()
    # packed per-partition constants: cols [0:NSKT) mask bias, [NSKT:NSKT+2) bq
    mbq_d = nc.dram_tensor("mbq", [128, NSKT + 2], FP, kind="ExternalInput").ap()
    ones_d = nc.dram_tensor("ones", [1, 128], FR, kind="ExternalInput").ap()
    idn_d = nc.dram_tensor("idn", [128, 128], BF, kind="ExternalInput").ap()
    out_d = nc.dram_tensor("out", [SQ, SQ], BF, kind="ExternalOutput").ap()

    def blocks_of(width, step=512):
        out, off = [], 0
        while off < width:
            w = min(step, width - off)
            out.append((off, w))
            off += w
        return out

    blocks = blocks_of(skp)

    with tile.TileContext(nc) as tc:
        # ---- resident tensors (one bufs=1 pool; distinct names = own slots) ----
        cpool_cm = tc.tile_pool(name="const", bufs=1)
        cpool = cpool_cm.__enter__()
        wk_sb = cpool.tile([128, NKC, DH], BF, name="wk_sb")
        wqv_sb = cpool.tile([128, NKC, 2 * DH], BF, name="wqv_sb")
        qx_sb = cpool.tile([128, NKC, SQ], BF, name="qx_sb")
        idn_sb = cpool.tile([128, 128], BF, name="idn_sb")
        wo_sb = cpool.tile([128, 2, SQ], FR, name="wo_sb")
        mbq_sb = cpool.tile([128, NSKT + 2], FP, name="mbq_sb")
        ones_sb = cpool.tile([1, 128], FR, name="ones_sb")
        qT_sb = cpool.tile([128, 2, SQ], FR, name="qT_sb")
        kT_sb = cpool.tile([128, 2, skp], FR, name="kT_sb")
        vext_sb = cpool.tile([128, NSKT, 65 * HPC], BF, name="vext_sb")
        ctxT_sb = cpool.tile([128, 2, SQ], FR, name="ctxT_sb")
        o0_sb = cpool.tile([128, 8, SQ], BF, name="o0_sb")

        vv = vext_sb[:, :, :].rearrange("p s (h c) -> p s h c", c=65)
        mb = mbq_sb[:, 0:NSKT]
        wq_sb = wqv_sb[:, :, 0:DH]
        wv_sb = wqv_sb[:, :, DH:2 * DH]

        nc.sync.dma_start(out=wk_sb[:], in_=wk_d.rearrange("(kc p) n -> p kc n", p=128))

        with tc.tile_pool(name="pa", bufs=1, space="PSUM") as pa, \
             tc.tile_pool(name="sexp", bufs=CFG["sexp"]) as sexp, \
             tc.tile_pool(name="sout", bufs=CFG["sout"]) as sout, \
             tc.tile_pool(name="sin", bufs=CFG["sin"]) as sin:

            # vext ones columns (written once, before any v data lands)
            ones_fp = sout.tile([128, NSKT], FP, tag="onesfp", name="ones_fp")
            nc.vector.memset(ones_fp[:], 1.0)
            with nc.allow_low_precision(reason="bf16 v extension"):
                nc.vector.tensor_scalar_add(
                    vv[:, :, :, 64:65],
                    ones_fp[:, :, None, None].to_broadcast((128, NSKT, HPC, 1)),
                    0.0)

            def scores(h, skt, ps_dst, lo, w=512):
                t, r0 = h // 2, 64 * (h % 2)
                nc.tensor.matmul(
                    ps_dst[:, 0:w],
                    lhsT=kT_sb[r0:r0 + 64, t, skt * 128:(skt + 1) * 128],
                    rhs=qT_sb[r0:r0 + 64, t, lo:lo + w],
                    start=True, stop=True)

            def ctx(h, skt, es_src, acc, lo, w=512):
                nc.tensor.matmul(
                    acc[:, lo:lo + w],
                    lhsT=vv[:, skt, h, :],
                    rhs=es_src[:, 0:w],
                    start=(skt == 0), stop=(skt == NSKT - 1))

            def normalize(h, acc, psum_pool, shape, tag):
                # per-sq-half chain so downstream consumers of the first half
                # (e.g. P3 tiles 0-3) unblock sooner
                t, r0 = h // 2, 64 * (h % 2)
                rec = sout.tile([1, SQ], FR, tag="rec", name="rec")
                bc_sb = sout.tile([64, SQ], FP, tag="bc", name="bc_sb")
                for lo in range(0, SQ, 512):
                    with nc.allow_low_precision(reason="float32r storage"):
                        nc.vector.reciprocal(rec[:, lo:lo + 512],
                                             acc[64:65, lo:lo + 512])
                    ps_bc = psum_pool.tile(shape, FP, tag=tag, name="ps_bc")
                    nc.tensor.matmul(ps_bc[:64, 0:512],
                                     lhsT=ones_sb[0:1, 0:64],
                                     rhs=rec[0:1, lo:lo + 512],
                                     start=True, stop=True)
                    nc.vector.tensor_copy(bc_sb[:, lo:lo + 512],
                                          ps_bc[:64, 0:512])
                    with nc.allow_low_precision(reason="float32r storage"):
                        nc.vector.tensor_tensor(
                            ctxT_sb[r0:r0 + 64, t, lo:lo + 512],
                            acc[0:64, lo:lo + 512], bc_sb[:, lo:lo + 512], MUL)

            # ---- P1: heads {0, 1}; projections spread as PE filler ----
            with tc.tile_pool(name="pb", bufs=1, space="PSUM") as pb, \
                 tc.tile_pool(name="pk", bufs=1, space="PSUM") as pk, \
                 tc.tile_pool(name="ps1", bufs=3, space="PSUM") as ps1:

                stage = {}

                def emit_k_dma(b):
                    off, w = blocks[b]
                    xin = sin.tile([128, NKC, 512], BF, tag="sin", name="xin")
                    nc.sync.dma_start(
                        out=xin[:, :, 0:w],
                        in_=kx_d.rearrange("(kc p) n -> p kc n", p=128)[:, :, off:off + w])
                    stage[b] = [xin, None]

                def emit_v_dma(b):
                    off, w = blocks[b]
                    vin = sin.tile([128, NKC, 512], BF, tag="sin", name="vin")
                    nc.sync.dma_start(
                        out=vin[:, :, 0:w],
                        in_=vx_d.rearrange("(kc p) n -> p kc n", p=128)[:, :, off:off + w])
                    stage[b][1] = vin

                def emit_block_dma(b):
                    emit_k_dma(b)
                    emit_v_dma(b)

                kstate = {}

                def kproj_half(xin, off, w, t, half):
                    if half == 0:
                        kstate[(off, t)] = pk.tile([128, 512], FP, tag="k",
                                                   name="ps_k")
                    ps = kstate[(off, t)]
                    for kc in range(half * 4, half * 4 + 4):
                        nc.tensor.matmul(
                            ps[:, 0:w],
                            lhsT=wk_sb[:, kc, t * 128:(t + 1) * 128],
                            rhs=xin[:, kc, 0:w],
                            start=(kc == 0), stop=(kc == NKC - 1))
                    if half == 1:
                        with nc.allow_low_precision(reason="float32r storage"):
                            nc.vector.tensor_scalar_add(
                                kT_sb[:, t, off:off + w], ps[:, 0:w], 0.0)

                def qproj(xin, off, w, t):
                    ps = pk.tile([128, 512], FP, tag="k", name="ps_q")
                    for kc in range(NKC):
                        nc.tensor.matmul(
                            ps[:, 0:w],
                            lhsT=wq_sb[:, kc, t * 128:(t + 1) * 128],
                            rhs=xin[:, kc, off:off + w],
                            start=(kc == 0), stop=(kc == NKC - 1))
                    with nc.allow_low_precision(reason="float32r storage"):
                        nc.vector.tensor_scalar_add(
                            qT_sb[:, t, off:off + w], ps[:, 0:w],
                            mbq_sb[:, NSKT + t:NSKT + t + 1])

                def vproj(vin, off, s):
                    skt = (off + s) // 128
                    ps_v = pk.tile([128, 512], FP, tag="k", name="ps_v")[:, 0:DH]
                    for kc in range(NKC):
                        nc.tensor.matmul(
                            ps_v[:, :],
                            lhsT=vin[:, kc, s:s + 128],
                            rhs=wv_sb[:, kc, :],
                            start=(kc == 0), stop=(kc == NKC - 1))
                    with nc.allow_low_precision(reason="bf16 v"):
                        nc.vector.tensor_scalar_add(
                            vv[:, skt, :, 0:64],
                            ps_v[:, :].rearrange("p (h c) -> p h c", c=64), 0.0)

                def proj_groups(b):
                    off, w = blocks[b]
                    xin, vin = stage[b]
                    gs = []
                    for half in range(2):
                        gs.append(lambda h=half: kproj_half(xin, off, w, 0, h))
                    for s in range(0, w, 128):
                        gs.append(lambda s=s: vproj(vin, off, s))
                    for half in range(2):
                        gs.append(lambda h=half: kproj_half(xin, off, w, 1, h))
                    return gs

                # prologue DMA: K weights + block-0 keys first (shortest path
                # to PE work), then QV weights, Q input; block-0 values last
                # (first needed by the ctx of key-tile 0, well into P1)
                emit_k_dma(0)
                nc.sync.dma_start(out=wqv_sb[:],
                                  in_=wqv_d.rearrange("(kc p) n -> p kc n", p=128))
                nc.sync.dma_start(out=mbq_sb[:], in_=mbq_d[:, :])
                qre = qT_d.rearrange("(kc p) n -> p kc n", p=128)
                nc.sync.dma_start(out=qx_sb[:, :, 0:512], in_=qre[:, :, 0:512])
                nc.sync.dma_start(out=qx_sb[:, :, 512:1024], in_=qre[:, :, 512:1024])
                emit_v_dma(0)

                # prologue PE: t=0 projections only -- heads 0,1 are both in
                # t-tile 0; all t=1 projection work rides the filler stream
                # and must only complete before P2a's first scores
                xin0, vin0 = stage[0]
                for half in range(2):
                    kproj_half(xin0, 0, 512, 0, half)
                for off, w in blocks_of(SQ):
                    qproj(qx_sb, off, w, 0)

                acc0 = pa.tile([65, SQ], FP, tag="acc", name="acc")
                acc1 = pb.tile([65, SQ], FP, tag="acc1", name="acc1")

                fillers = [lambda s=s: vproj(vin0, 0, s) for s in range(0, 512, 128)]
                fillers += [lambda off=off, w=w: qproj(qx_sb, off, w, 1)
                            for off, w in blocks_of(SQ)]
                fillers += [lambda h=h: kproj_half(xin0, 0, 512, 1, h)
                            for h in range(2)]
                p1_pend = []

                def fill():
                    if fillers:
                        fillers.pop(0)()

                for b, (off, w) in enumerate(blocks):
                    if b + 1 < len(blocks):
                        emit_block_dma(b + 1)
                        fillers.extend(proj_groups(b + 1))
                    if b == 1:
                        nc.sync.dma_start(out=ones_sb[:], in_=ones_d[:, :])
                        nc.sync.dma_start(out=idn_sb[:], in_=idn_d[:, :])
                        nc.sync.dma_start(
                            out=wo_sb[:],
                            in_=wo_d.rearrange("(t p) n -> p t n", p=128))
                    for s in range(0, w, 128):
                        skt = (off + s) // 128
                        for h in range(2):
                            for lo in (0, 512):
                                ps_s = ps1.tile([128, 512], FP, tag="s",
                                                name="ps_s")
                                scores(h, skt, ps_s, lo)
                                es = sexp.tile([128, 512], BF, tag="es5",
                                               name="es5")
                                nc.scalar.activation(
                                    es[:, :], ps_s[:, :], AF.Exp,
                                    bias=mb[:, skt:skt + 1], scale=SCALE)
                                p1_pend.append((h, skt, lo, es))
                                if (h, lo) != (0, 0):
                                    fill()
                                if len(p1_pend) > 2:
                                    ph, pskt, plo, pes = p1_pend.pop(0)
                                    ctx(ph, pskt, pes,
                                        acc0 if ph == 0 else acc1, plo)

                while fillers:
                    fillers.pop(0)()
                for ph, pskt, plo, pes in p1_pend:
                    ctx(ph, pskt, pes, acc0 if ph == 0 else acc1, plo)
                normalize(0, acc0, ps1, [128, 512], "s")
                normalize(1, acc1, ps1, [128, 512], "s")

            # ---- P2a: head 2 + t=0 half of the output projection ----
            with tc.tile_pool(name="ps2", bufs=2, space="PSUM") as ps2, \
                 tc.tile_pool(name="po0", bufs=2, space="PSUM") as po0:
                acc2 = pa.tile([65, SQ], FP, tag="acc", name="acc")

                def out_t0(sq, lo):
                    ps_o = po0.tile([128, 512], FP, tag="o0", name="ps_o0")
                    nc.tensor.matmul(
                        ps_o[:, :],
                        lhsT=ctxT_sb[:, 0, sq * 128:(sq + 1) * 128],
                        rhs=wo_sb[:, 0, lo:lo + 512],
                        start=True, stop=True)
                    with nc.allow_low_precision(reason="bf16 staging"):
                        nc.vector.tensor_scalar_add(o0_sb[:, sq, lo:lo + 512],
                                                    ps_o[:, :], 0.0)

                pend = []
                ohalf = 0
                for skt in range(NSKT):
                    ps_s = ps2.tile([128, SQ], FP, tag="s2", name="ps_p2")
                    scores(2, skt, ps_s[:, 0:512], 0)
                    scores(2, skt, ps_s[:, 512:1024], 512)
                    es = sexp.tile([128, SQ], BF, tag="es", name="es")
                    nc.scalar.activation(
                        es[:, :], ps_s[:, :], AF.Exp,
                        bias=mb[:, skt:skt + 1], scale=SCALE)
                    pend.append((skt, es))
                    if ohalf < 16:
                        out_t0(ohalf // 2, 512 * (ohalf % 2))
                        ohalf += 1
                    if len(pend) > 2:
                        pskt, pes = pend.pop(0)
                        ctx(2, pskt, pes[:, 0:512], acc2, 0)
                        ctx(2, pskt, pes[:, 512:1024], acc2, 512)
                while ohalf < 16:
                    out_t0(ohalf // 2, 512 * (ohalf % 2))
                    ohalf += 1
                for pskt, pes in pend:
                    ctx(2, pskt, pes[:, 0:512], acc2, 0)
                    ctx(2, pskt, pes[:, 512:1024], acc2, 512)

                # ---- P2b: head 3; head-2 normalize folded in ----
                acc3 = pa.tile([65, SQ], FP, tag="acc", name="acc")
                pend = []
                for skt in range(NSKT):
                    ps_s = ps2.tile([128, SQ], FP, tag="s2", name="ps_p2")
                    scores(3, skt, ps_s[:, 0:512], 0)
                    scores(3, skt, ps_s[:, 512:1024], 512)
                    es = sexp.tile([128, SQ], BF, tag="es", name="es")
                    nc.scalar.activation(
                        es[:, :], ps_s[:, :], AF.Exp,
                        bias=mb[:, skt:skt + 1], scale=SCALE)
                    pend.append((skt, es))
                    if skt == 1:
                        normalize(2, acc2, po0, [128, 512], "o0")
                    if len(pend) > 2:
                        pskt, pes = pend.pop(0)
                        ctx(3, pskt, pes[:, 0:512], acc3, 0)
                        ctx(3, pskt, pes[:, 512:1024], acc3, 512)
                for pskt, pes in pend:
                    ctx(3, pskt, pes[:, 0:512], acc3, 0)
                    ctx(3, pskt, pes[:, 512:1024], acc3, 512)
                normalize(3, acc3, po0, [128, 512], "o0")

                # ---- P3: out = staged t0 (identity-matmul, independent of
                # the head-3 normalize) + t1 matmul; out staged in one SBUF
                # buffer and shipped as two large DMAs ----
                ob_sb = cpool.tile([128, 8, SQ], BF, name="ob_sb")
                for sq in range(SQ // 128):
                    ps_o = ps2.tile([128, SQ], FP, tag="s2", name="ps_o")
                    for lo in range(0, SQ, 512):
                        nc.tensor.matmul(
                            ps_o[:, lo:lo + 512],
                            lhsT=idn_sb[:, :],
                            rhs=o0_sb[:, sq, lo:lo + 512],
                            start=True, stop=False)
                        nc.tensor.matmul(
                            ps_o[:, lo:lo + 512],
                            lhsT=ctxT_sb[:, 1, sq * 128:(sq + 1) * 128],
                            rhs=wo_sb[:, 1, lo:lo + 512],
                            start=False, stop=True)
                    if sq % 2 == 0:
                        nc.scalar.activation(ob_sb[:, sq, :], ps_o[:], AF.Copy)
                    else:
                        with nc.allow_low_precision(reason="bf16 output"):
                            nc.vector.tensor_scalar_add(ob_sb[:, sq, :], ps_o[:], 0.0)
                    if sq == 3:
                        nc.sync.dma_start(
                            out=out_d.rearrange("(t p) n -> p t n", p=128)[:, 0:4, :],
                            in_=ob_sb[:, 0:4, :])
                nc.sync.dma_start(
                    out=out_d.rearrange("(t p) n -> p t n", p=128)[:, 4:8, :],
                    in_=ob_sb[:, 4:8, :])

        cpool_cm.__exit__(None, None, None)

    nc.compile()
    return nc


def get_nc(skp=SK):
    key = ("nc", skp)
    if key not in _CACHE:
        _CACHE[key] = _build(skp)
    return _CACHE[key]


def make_in_maps(query, key, value, key_mask, Wq, bq, Wk, bk, Wv, bv, Wo, bo):
    f32 = lambda x: np.asarray(x, dtype=np.float32)
    bf16 = lambda x: np.ascontiguousarray(np.asarray(x, np.float32).astype(ml_dtypes.bfloat16))
    query, key, value = f32(query), f32(key), f32(value)
    Wq, bq, Wk = f32(Wq), f32(bq), f32(Wk)
    Wv, bv, Wo, bo = f32(Wv), f32(bv), f32(Wo), f32(bo)
    key_mask = np.asarray(key_mask)

    # compact unmasked keys; pad to a common multiple of 128
    keep = [np.nonzero(key_mask[b] != 0)[0] for b in range(B)]
    skp = max(512, int(-(-max(len(k) for k in keep) // 128) * 128))
    skp = min(skp, SK)
    nskt = skp // 128

    # bk dropped (softmax shift-invariance); bv folded into the host-side
    # output bias:  out += (bv @ Wo);  bo handled on host too.
    bo_eff = bo + bv @ Wo

    qT, kxT, vxT, mb = [], [], [], []
    for b in range(B):
        n = len(keep[b])
        kc = np.zeros((skp, IN), np.float32)
        vc = np.zeros((skp, IN), np.float32)
        kc[:n] = key[b][keep[b]]
        vc[:n] = value[b][keep[b]]
        mbias = np.full(skp, -1e9, np.float32)
        mbias[:n] = 0.0
        qT.append(bf16(query[b].T))
        kxT.append(bf16(kc.T))
        vxT.append(bf16(vc.T))
        mb.append(np.ascontiguousarray(mbias.reshape(nskt, 128).T))

    in_maps = []
    for c in range(NCORES):
        b, g = c // 4, c % 4
        S = slice(DH * g, DH * (g + 1))
        mbq = np.concatenate(
            [mb[b], bq[S][0:128][:, None], bq[S][128:256][:, None]], axis=1)
        in_maps.append({
            "qT": qT[b], "kx": kxT[b], "vx": vxT[b],
            "wk": bf16(Wk[:, S]),
            "wqv": bf16(np.concatenate([Wq[:, S], Wv[:, S]], axis=1)),
            "wo": np.ascontiguousarray(Wo[S, :]),
            "mbq": np.ascontiguousarray(mbq),
            "ones": np.ones((1, 128), np.float32),
            "idn": np.eye(128, dtype=ml_dtypes.bfloat16),
        })
    return in_maps, skp, bo_eff


def run(in_maps, skp=SK, trace=False):
    from concourse.bass_utils import run_bass_kernel_spmd
    nc = get_nc(skp)
    res = run_bass_kernel_spmd(nc, in_maps, list(range(NCORES)), trace=trace)
    _CACHE["last_results"] = res
    return res


def kernel(query, key, value, key_mask, Wq, bq, Wk, bk, Wv, bv, Wo, bo):
    in_maps, skp, bo_eff = make_in_maps(query, key, value, key_mask,
                                        Wq, bq, Wk, bk, Wv, bv, Wo, bo)
    res = run(in_maps, skp)
    out = np.zeros((B, SQ, SQ), np.float32)
    for c in range(NCORES):
        out[c // 4] += np.asarray(res.results[c]["out"], np.float32)
    out += bo_eff[None, None, :]
    return out


# revision 31
# speedup vs baseline: 1.0593x; 1.0158x over previous
"""Multi-head cross-attention Trainium2 kernel.

Full-input contract: kernel(**inputs) takes the complete tensors and returns
the complete output. Internally shards over 8 NeuronCores as
(batch x head-group): core c handles batch c//4 and heads [4*(c%4), 4*(c%4)+4).
Each core computes its partial output  ctx_g @ Wo_g  for its batch; the host
sums the 4 head-group partials per batch and adds bo.

Masked keys (key_mask == 0) contribute exactly zero probability, so the host
compacts key/value to the unmasked rows (padded up to a multiple of 128 with
-1e9 score bias), which shrinks the K/V projections and the whole attention
core proportionally.

Numerics: HBM-resident activations and projection weights are bf16 (halves
the DMA-bound), everything on-chip is fp32 (fp32r for matmul operands, which
streams at full rate for moving dims >= 256). Exact algebraic cuts:
  - bk is dropped entirely: a per-query constant added to every key's score
    is softmax-invariant.
  - bv is folded into bo on the host: probs sum to 1, so ctx = P@(v + bv) =
    P@v + bv, hence out += bv @ Wo, absorbed into bo.
  - bq is applied on-device in the Q projection.

Engines execute their queues in emission order, so the emission is software
pipelined (PSUM is the scarce resource - 8 banks):
  P1  prologue: block-0 K projection + Q projection.  Then heads 0,1
      attention per 128-key tile; the remaining K/V projection matmul groups
      (value-chunk-stationary V proj lands directly in [sk, d] layout - no PE
      transpose) are spread between dependent score/exp/ctx ops as PE filler
      to hide the ~800ns PSUM-drain+semaphore latency around each exp.
  P2a head 2 attention (one 1024-wide exp per key tile, ctx lagging 2 units
      so the pipeline is ACT-rate-limited), interleaved with the t=0 half of
      the output projection (heads 0,1 context ready) into 2 spare PSUM
      banks, staged to SBUF in fp32.
  P2b head 3 attention, with head-2 normalization folded in.
  P3  out = t=1 half + staged t=0 half, written back as bf16; host upcasts
      and reduces.
"""

import numpy as np
import ml_dtypes

B, SQ, SK, IN = 2, 1024, 4096, 1024
H_TOT, D, HPC = 16, 64, 4
DH = HPC * D  # 256, per-core head-dim slice
NCORES = 8

_CACHE = {}

# scheduling knobs
CFG = {"sin": 4, "sexp": 6, "sout": 3, "fillat": (2, 3)}


def _build(skp):
    import concourse.tile as tile
    from concourse import bacc, mybir

    FP = mybir.dt.float32
    FR = mybir.dt.float32r
    BF = mybir.dt.bfloat16
    AF = mybir.ActivationFunctionType
    MUL = mybir.AluOpType.mult
    ADD = mybir.AluOpType.add

    nc = bacc.Bacc("TRN2", target_bir_lowering=False, debug=False)

    NSKT = skp // 128          # sk tiles of 128
    NKC = IN // 128            # 8 contraction chunks
    SCALE = 1.0 / float(np.sqrt(D))

    qT_d = nc.dram_tensor("qT", [IN, SQ], BF, kind="ExternalInput").ap()
    kx_d = nc.dram_tensor("kx", [IN, skp], BF, kind="ExternalInput").ap()
    vx_d = nc.dram_tensor("vx", [IN, skp], BF, kind="ExternalInput").ap()
    wk_d = nc.dram_tensor("wk", [IN, DH], BF, kind="ExternalInput").ap()
    wqv_d = nc.dram_tensor("wqv", [IN, 2 * DH], BF, kind="ExternalInput").ap()
    wo_d = nc.dram_tensor("wo", [DH, SQ], FR, kind="ExternalInput").ap()
    # packed per-partition constants: cols [0:NSKT) mask bias, [NSKT:NSKT+2) bq
    mbq_d = nc.dram_tensor("mbq", [128, NSKT + 2], FP, kind="ExternalInput").ap()
    ones_d = nc.dram_tensor("ones", [1, 128], FR, kind="ExternalInput").ap()
    idn_d = nc.dram_tensor("idn", [128, 128], BF, kind="ExternalInput").ap()
    out_d = nc.dram_tensor("out", [SQ, SQ], BF, kind="ExternalOutput").ap()

    def blocks_of(width, step=512):
        out, off = [], 0
        while off < width:
            w = min(step, width - off)
            out.append((off, w))
            off += w
        return out

    blocks = blocks_of(skp)

    with tile.TileContext(nc) as tc:
        # ---- resident tensors (one bufs=1 pool; distinct names = own slots) ----
        cpool_cm = tc.tile_pool(name="const", bufs=1)
        cpool = cpool_cm.__enter__()
        wk_sb = cpool.tile([128, NKC, DH], BF, name="wk_sb")
        wqv_sb = cpool.tile([128, NKC, 2 * DH], BF, name="wqv_sb")
        qx_sb = cpool.tile([128, NKC, SQ], BF, name="qx_sb")
        idn_sb = cpool.tile([128, 128], BF, name="idn_sb")
        wo_sb = cpool.tile([128, 2, SQ], FR, name="wo_sb")
        mbq_sb = cpool.tile([128, NSKT + 2], FP, name="mbq_sb")
        ones_sb = cpool.tile([1, 128], FR, name="ones_sb")
        qT_sb = cpool.tile([128, 2, SQ], FR, name="qT_sb")
        kT_sb = cpool.tile([128, 2, skp], FR, name="kT_sb")
        vext_sb = cpool.tile([128, NSKT, 65 * HPC], BF, name="vext_sb")
        ctxT_sb = cpool.tile([128, 2, SQ], FR, name="ctxT_sb")
        o0_sb = cpool.tile([128, 8, SQ], BF, name="o0_sb")

        vv = vext_sb[:, :, :].rearrange("p s (h c) -> p s h c", c=65)
        mb = mbq_sb[:, 0:NSKT]
        wq_sb = wqv_sb[:, :, 0:DH]
        wv_sb = wqv_sb[:, :, DH:2 * DH]

        nc.sync.dma_start(out=wk_sb[:], in_=wk_d.rearrange("(kc p) n -> p kc n", p=128))

        with tc.tile_pool(name="pa", bufs=1, space="PSUM") as pa, \
             tc.tile_pool(name="sexp", bufs=CFG["sexp"]) as sexp, \
             tc.tile_pool(name="sout", bufs=CFG["sout"]) as sout, \
             tc.tile_pool(name="sin", bufs=CFG["sin"]) as sin:

            # vext ones columns (written once, before any v data lands)
            ones_fp = sout.tile([128, NSKT], FP, tag="onesfp", name="ones_fp")
            nc.vector.memset(ones_fp[:], 1.0)
            with nc.allow_low_precision(reason="bf16 v extension"):
                nc.vector.tensor_scalar_add(
                    vv[:, :, :, 64:65],
                    ones_fp[:, :, None, None].to_broadcast((128, NSKT, HPC, 1)),
                    0.0)

            def scores(h, skt, ps_dst, lo, w=512):
                t, r0 = h // 2, 64 * (h % 2)
                nc.tensor.matmul(
                    ps_dst[:, 0:w],
                    lhsT=kT_sb[r0:r0 + 64, t, skt * 128:(skt + 1) * 128],
                    rhs=qT_sb[r0:r0 + 64, t, lo:lo + w],
                    start=True, stop=True)

            def ctx(h, skt, es_src, acc, lo, w=512):
                nc.tensor.matmul(
                    acc[:, lo:lo + w],
                    lhsT=vv[:, skt, h, :],
                    rhs=es_src[:, 0:w],
                    start=(skt == 0), stop=(skt == NSKT - 1))

            def normalize(h, acc, psum_pool, shape, tag):
                # per-sq-half chain so downstream consumers of the first half
                # (e.g. P3 tiles 0-3) unblock sooner
                t, r0 = h // 2, 64 * (h % 2)
                rec = sout.tile([1, SQ], FR, tag="rec", name="rec")
                bc_sb = sout.tile([64, SQ], FP, tag="bc", name="bc_sb")
                for lo in range(0, SQ, 512):
                    with nc.allow_low_precision(reason="float32r storage"):
                        nc.vector.reciprocal(rec[:, lo:lo + 512],
                                             acc[64:65, lo:lo + 512])
                    ps_bc = psum_pool.tile(shape, FP, tag=tag, name="ps_bc")
                    nc.tensor.matmul(ps_bc[:64, 0:512],
                                     lhsT=ones_sb[0:1, 0:64],
                                     rhs=rec[0:1, lo:lo + 512],
                                     start=True, stop=True)
                    nc.vector.tensor_copy(bc_sb[:, lo:lo + 512],
                                          ps_bc[:64, 0:512])
                    with nc.allow_low_precision(reason="float32r storage"):
                        nc.vector.tensor_tensor(
                            ctxT_sb[r0:r0 + 64, t, lo:lo + 512],
                            acc[0:64, lo:lo + 512], bc_sb[:, lo:lo + 512], MUL)

            # ---- P1: heads {0, 1}; projections spread as PE filler ----
            with tc.tile_pool(name="pb", bufs=1, space="PSUM") as pb, \
                 tc.tile_pool(name="pk", bufs=1, space="PSUM") as pk, \
                 tc.tile_pool(name="ps1", bufs=3, space="PSUM") as ps1:

                stage = {}

                def emit_k_dma(b):
                    off, w = blocks[b]
                    xin = sin.tile([128, NKC, 512], BF, tag="sin", name="xin")
                    nc.sync.dma_start(
                        out=xin[:, :, 0:w],
                        in_=kx_d.rearrange("(kc p) n -> p kc n", p=128)[:, :, off:off + w])
                    stage[b] = [xin, None]

                def emit_v_dma(b):
                    off, w = blocks[b]
                    vin = sin.tile([128, NKC, 512], BF, tag="sin", name="vin")
                    nc.sync.dma_start(
                        out=vin[:, :, 0:w],
                        in_=vx_d.rearrange("(kc p) n -> p kc n", p=128)[:, :, off:off + w])
                    stage[b][1] = vin

                def emit_block_dma(b):
                    emit_k_dma(b)
                    emit_v_dma(b)

                kstate = {}

                def kproj_half(xin, off, w, t, half):
                    if half == 0:
                        kstate[(off, t)] = pk.tile([128, 512], FP, tag="k",
                                                   name="ps_k")
                    ps = kstate[(off, t)]
                    for kc in range(half * 4, half * 4 + 4):
                        nc.tensor.matmul(
                            ps[:, 0:w],
                            lhsT=wk_sb[:, kc, t * 128:(t + 1) * 128],
                            rhs=xin[:, kc, 0:w],
                            start=(kc == 0), stop=(kc == NKC - 1))
                    if half == 1:
                        with nc.allow_low_precision(reason="float32r storage"):
                            nc.vector.tensor_scalar_add(
                                kT_sb[:, t, off:off + w], ps[:, 0:w], 0.0)

                def qproj(xin, off, w, t):
                    ps = pk.tile([128, 512], FP, tag="k", name="ps_q")
                    for kc in range(NKC):
                        nc.tensor.matmul(
                            ps[:, 0:w],
                            lhsT=wq_sb[:, kc, t * 128:(t + 1) * 128],
                            rhs=xin[:, kc, off:off + w],
                            start=(kc == 0), stop=(kc == NKC - 1))
                    with nc.allow_low_precision(reason="float32r storage"):
                        nc.vector.tensor_scalar_add(
                            qT_sb[:, t, off:off + w], ps[:, 0:w],
                            mbq_sb[:, NSKT + t:NSKT + t + 1])

                def vproj(vin, off, s):
                    skt = (off + s) // 128
                    ps_v = pk.tile([128, 512], FP, tag="k", name="ps_v")[:, 0:DH]
                    for kc in range(NKC):
                        nc.tensor.matmul(
                            ps_v[:, :],
                            lhsT=vin[:, kc, s:s + 128],
                            rhs=wv_sb[:, kc, :],
                            start=(kc == 0), stop=(kc == NKC - 1))
                    with nc.allow_low_precision(reason="bf16 v"):
                        nc.vector.tensor_scalar_add(
                            vv[:, skt, :, 0:64],
                            ps_v[:, :].rearrange("p (h c) -> p h c", c=64), 0.0)

                def proj_groups(b):
                    off, w = blocks[b]
                    xin, vin = stage[b]
                    gs = []
                    for half in range(2):
                        gs.append(lambda h=half: kproj_half(xin, off, w, 0, h))
                    for s in range(0, w, 128):
                        gs.append(lambda s=s: vproj(vin, off, s))
                    for half in range(2):
                        gs.append(lambda h=half: kproj_half(xin, off, w, 1, h))
                    return gs

                # prologue DMA: K weights + block-0 keys first (shortest path
                # to PE work), then QV weights, Q input; block-0 values last
                # (first needed by the ctx of key-tile 0, well into P1)
                emit_k_dma(0)
                nc.sync.dma_start(out=wqv_sb[:],
                                  in_=wqv_d.rearrange("(kc p) n -> p kc n", p=128))
                nc.sync.dma_start(out=mbq_sb[:], in_=mbq_d[:, :])
                qre = qT_d.rearrange("(kc p) n -> p kc n", p=128)
                nc.sync.dma_start(out=qx_sb[:, :, 0:512], in_=qre[:, :, 0:512])
                nc.sync.dma_start(out=qx_sb[:, :, 512:1024], in_=qre[:, :, 512:1024])
                emit_v_dma(0)

                # prologue PE: t=0 projections only -- heads 0,1 are both in
                # t-tile 0; all t=1 projection work rides the filler stream
                # and must only complete before P2a's first scores
                xin0, vin0 = stage[0]
                for half in range(2):
                    kproj_half(xin0, 0, 512, 0, half)
                for off, w in blocks_of(SQ):
                    qproj(qx_sb, off, w, 0)

                acc0 = pa.tile([65, SQ], FP, tag="acc", name="acc")
                acc1 = pb.tile([65, SQ], FP, tag="acc1", name="acc1")

                fillers = [lambda s=s: vproj(vin0, 0, s) for s in range(0, 512, 128)]
                fillers += [lambda off=off, w=w: qproj(qx_sb, off, w, 1)
                            for off, w in blocks_of(SQ)]
                fillers += [lambda h=h: kproj_half(xin0, 0, 512, 1, h)
                            for h in range(2)]
                p1_pend = []

                def fill():
                    if fillers:
                        fillers.pop(0)()

                for b, (off, w) in enumerate(blocks):
                    if b + 1 < len(blocks):
                        emit_block_dma(b + 1)
                        fillers.extend(proj_groups(b + 1))
                    if b == 1:
                        nc.sync.dma_start(out=ones_sb[:], in_=ones_d[:, :])
                        nc.sync.dma_start(out=idn_sb[:], in_=idn_d[:, :])
                        nc.sync.dma_start(
                            out=wo_sb[:],
                            in_=wo_d.rearrange("(t p) n -> p t n", p=128))
                    for s in range(0, w, 128):
                        skt = (off + s) // 128
                        for h in range(2):
                            for lo in (0, 512):
                                ps_s = ps1.tile([128, 512], FP, tag="s",
                                                name="ps_s")
                                scores(h, skt, ps_s, lo)
                                es = sexp.tile([128, 512], BF, tag="es5",
                                               name="es5")
                                nc.scalar.activation(
                                    es[:, :], ps_s[:, :], AF.Exp,
                                    bias=mb[:, skt:skt + 1], scale=SCALE)
                                p1_pend.append((h, skt, lo, es))
                                if (h * 2 + lo // 512) in CFG["fillat"]:
                                    fill()
                                if len(p1_pend) > 2:
                                    ph, pskt, plo, pes = p1_pend.pop(0)
                                    ctx(ph, pskt, pes,
                                        acc0 if ph == 0 else acc1, plo)

                while fillers:
                    fillers.pop(0)()
                for ph, pskt, plo, pes in p1_pend:
                    ctx(ph, pskt, pes, acc0 if ph == 0 else acc1, plo)
                normalize(0, acc0, ps1, [128, 512], "s")
                normalize(1, acc1, ps1, [128, 512], "s")

            # ---- P2a: head 2 + t=0 half of the output projection ----
            with tc.tile_pool(name="ps2", bufs=2, space="PSUM") as ps2, \
                 tc.tile_pool(name="po0", bufs=2, space="PSUM") as po0:
                acc2 = pa.tile([65, SQ], FP, tag="acc", name="acc")

                def out_t0(sq, lo):
                    ps_o = po0.tile([128, 512], FP, tag="o0", name="ps_o0")
                    nc.tensor.matmul(
                        ps_o[:, :],
                        lhsT=ctxT_sb[:, 0, sq * 128:(sq + 1) * 128],
                        rhs=wo_sb[:, 0, lo:lo + 512],
                        start=True, stop=True)
                    with nc.allow_low_precision(reason="bf16 staging"):
                        nc.vector.tensor_scalar_add(o0_sb[:, sq, lo:lo + 512],
                                                    ps_o[:, :], 0.0)

                pend = []
                ohalf = 0
                for skt in range(NSKT):
                    ps_s = ps2.tile([128, SQ], FP, tag="s2", name="ps_p2")
                    scores(2, skt, ps_s[:, 0:512], 0)
                    scores(2, skt, ps_s[:, 512:1024], 512)
                    es = sexp.tile([128, SQ], BF, tag="es", name="es")
                    nc.scalar.activation(
                        es[:, :], ps_s[:, :], AF.Exp,
                        bias=mb[:, skt:skt + 1], scale=SCALE)
                    pend.append((skt, es))
                    if ohalf < 16:
                        out_t0(ohalf // 2, 512 * (ohalf % 2))
                        ohalf += 1
                    if len(pend) > 2:
                        pskt, pes = pend.pop(0)
                        ctx(2, pskt, pes[:, 0:512], acc2, 0)
                        ctx(2, pskt, pes[:, 512:1024], acc2, 512)
                while ohalf < 16:
                    out_t0(ohalf // 2, 512 * (ohalf % 2))
                    ohalf += 1
                for pskt, pes in pend:
                    ctx(2, pskt, pes[:, 0:512], acc2, 0)
                    ctx(2, pskt, pes[:, 512:1024], acc2, 512)

                # ---- P2b: head 3; head-2 normalize folded in ----
                acc3 = pa.tile([65, SQ], FP, tag="acc", name="acc")
                pend = []
                for skt in range(NSKT):
                    ps_s = ps2.tile([128, SQ], FP, tag="s2", name="ps_p2")
                    scores(3, skt, ps_s[:, 0:512], 0)
                    scores(3, skt, ps_s[:, 512:1024], 512)
                    es = sexp.tile([128, SQ], BF, tag="es", name="es")
                    nc.scalar.activation(
                        es[:, :], ps_s[:, :], AF.Exp,
                        bias=mb[:, skt:skt + 1], scale=SCALE)
                    pend.append((skt, es))
                    if skt == 1:
                        normalize(2, acc2, po0, [128, 512], "o0")
                    if len(pend) > 2:
                        pskt, pes = pend.pop(0)
                        ctx(3, pskt, pes[:, 0:512], acc3, 0)
                        ctx(3, pskt, pes[:, 512:1024], acc3, 512)
                for pskt, pes in pend:
                    ctx(3, pskt, pes[:, 0:512], acc3, 0)
                    ctx(3, pskt, pes[:, 512:1024], acc3, 512)
                normalize(3, acc3, po0, [128, 512], "o0")

                # ---- P3: out = staged t0 (identity-matmul, independent of
                # the head-3 normalize) + t1 matmul; out staged in one SBUF
                # buffer and shipped as two large DMAs ----
                ob_sb = cpool.tile([128, 8, SQ], BF, name="ob_sb")
                for sq in range(SQ // 128):
                    ps_o = ps2.tile([128, SQ], FP, tag="s2", name="ps_o")
                    for lo in range(0, SQ, 512):
                        nc.tensor.matmul(
                            ps_o[:, lo:lo + 512],
                            lhsT=idn_sb[:, :],
                            rhs=o0_sb[:, sq, lo:lo + 512],
                            start=True, stop=False)
                        nc.tensor.matmul(
                            ps_o[:, lo:lo + 512],
                            lhsT=ctxT_sb[:, 1, sq * 128:(sq + 1) * 128],
                            rhs=wo_sb[:, 1, lo:lo + 512],
                            start=False, stop=True)
                    if sq % 2 == 0:
                        nc.scalar.activation(ob_sb[:, sq, :], ps_o[:], AF.Copy)
                    else:
                        with nc.allow_low_precision(reason="bf16 output"):
                            nc.vector.tensor_scalar_add(ob_sb[:, sq, :], ps_o[:], 0.0)
                    if sq == 3:
                        nc.sync.dma_start(
                            out=out_d.rearrange("(t p) n -> p t n", p=128)[:, 0:4, :],
                            in_=ob_sb[:, 0:4, :])
                nc.sync.dma_start(
                    out=out_d.rearrange("(t p) n -> p t n", p=128)[:, 4:8, :],
                    in_=ob_sb[:, 4:8, :])

        cpool_cm.__exit__(None, None, None)

    nc.compile()
    return nc


def get_nc(skp=SK):
    key = ("nc", skp)
    if key not in _CACHE:
        _CACHE[key] = _build(skp)
    return _CACHE[key]


def make_in_maps(query, key, value, key_mask, Wq, bq, Wk, bk, Wv, bv, Wo, bo):
    f32 = lambda x: np.asarray(x, dtype=np.float32)
    bf16 = lambda x: np.ascontiguousarray(np.asarray(x, np.float32).astype(ml_dtypes.bfloat16))
    query, key, value = f32(query), f32(key), f32(value)
    Wq, bq, Wk = f32(Wq), f32(bq), f32(Wk)
    Wv, bv, Wo, bo = f32(Wv), f32(bv), f32(Wo), f32(bo)
    key_mask = np.asarray(key_mask)

    # compact unmasked keys; pad to a common multiple of 128
    keep = [np.nonzero(key_mask[b] != 0)[0] for b in range(B)]
    skp = max(512, int(-(-max(len(k) for k in keep) // 128) * 128))
    skp = min(skp, SK)
    nskt = skp // 128

    # bk dropped (softmax shift-invariance); bv folded into the host-side
    # output bias:  out += (bv @ Wo);  bo handled on host too.
    bo_eff = bo + bv @ Wo

    qT, kxT, vxT, mb = [], [], [], []
    for b in range(B):
        n = len(keep[b])
        kc = np.zeros((skp, IN), np.float32)
        vc = np.zeros((skp, IN), np.float32)
        kc[:n] = key[b][keep[b]]
        vc[:n] = value[b][keep[b]]
        mbias = np.full(skp, -1e9, np.float32)
        mbias[:n] = 0.0
        qT.append(bf16(query[b].T))
        kxT.append(bf16(kc.T))
        vxT.append(bf16(vc.T))
        mb.append(np.ascontiguousarray(mbias.reshape(nskt, 128).T))

    in_maps = []
    for c in range(NCORES):
        b, g = c // 4, c % 4
        S = slice(DH * g, DH * (g + 1))
        mbq = np.concatenate(
            [mb[b], bq[S][0:128][:, None], bq[S][128:256][:, None]], axis=1)
        in_maps.append({
            "qT": qT[b], "kx": kxT[b], "vx": vxT[b],
            "wk": bf16(Wk[:, S]),
            "wqv": bf16(np.concatenate([Wq[:, S], Wv[:, S]], axis=1)),
            "wo": np.ascontiguousarray(Wo[S, :]),
            "mbq": np.ascontiguousarray(mbq),
            "ones": np.ones((1, 128), np.float32),
            "idn": np.eye(128, dtype=ml_dtypes.bfloat16),
        })
    return in_maps, skp, bo_eff


def run(in_maps, skp=SK, trace=False):
    from concourse.bass_utils import run_bass_kernel_spmd
    nc = get_nc(skp)
    res = run_bass_kernel_spmd(nc, in_maps, list(range(NCORES)), trace=trace)
    _CACHE["last_results"] = res
    return res


def kernel(query, key, value, key_mask, Wq, bq, Wk, bk, Wv, bv, Wo, bo):
    in_maps, skp, bo_eff = make_in_maps(query, key, value, key_mask,
                                        Wq, bq, Wk, bk, Wv, bv, Wo, bo)
    res = run(in_maps, skp)
    out = np.zeros((B, SQ, SQ), np.float32)
    for c in range(NCORES):
        out[c // 4] += np.asarray(res.results[c]["out"], np.float32)
    out += bo_eff[None, None, :]
    return out
